# revision 1
# baseline (speedup 1.0000x reference)
"""GAT (2-layer) + global pooling Trainium2 kernel, 8-way SPMD.

Sharding: destination-node sharding for both GAT layers. Each core owns a
contiguous range of 6250 destination nodes and all edges incident to them.
Per-core node permutation puts owned nodes at table rows [0, 6250) so the
program is identical across cores (pure SPMD, data-only divergence).

Layer 1 node transform (h1 = x @ W1.T, plus attention scalars folded into the
weight matrix) is computed redundantly on every core; the layer-2 node
transform is sharded and all-gathered (narrow [6250, 66] rows).

Per-edge work uses dma_gather (GPSIMD ucode) to fetch h|a_src rows by source
index; segment softmax over dst is computed without the max-subtraction
(mathematically identical, values are small) by accumulating exp-weighted
numerators and denominators in PSUM via a selection-matrix matmul per
128-edge tile.
"""
import sys
import numpy as np

for _p in ("/opt/trn_rl_repo", "/root/.axon_site"):
    if _p not in sys.path:
        sys.path.insert(0, _p)

import concourse.bass as bass
import concourse.bacc as bacc
import concourse.tile as tile
import concourse.mybir as mybir
from concourse import bass_utils, library_config

F32 = mybir.dt.float32
BF16 = mybir.dt.bfloat16
I16 = mybir.dt.int16

NCORES = 8
NEG_SLOPE = 0.2
CALL_TILES = 8          # max tiles (of 128 edges) per dma_gather call

# layer-1 table row: [h1 (256) | a1s (4) | a1d (4) | pad] -> 320 f32 = 1280 B
T1_H, T1_AS, T1_AD, T1_COLS = 256, 256, 260, 320
# layer-2 table row: [h2 (64) | a2s (1) | a2d (1) | pad] -> 128 f32 = 512 B
T2_H, T2_AS, T2_COLS = 64, 64, 128
PK_COLS = 66            # allgather pack row: h2 | a2s | a2d


def _ceil_div(a, b):
    return -(-a // b)


def _call_sizes(ntiles):
    """Split ntiles into dma_gather calls of at most CALL_TILES tiles."""
    out = []
    while ntiles > 0:
        c = min(ntiles, CALL_TILES)
        out.append(c)
        ntiles -= c
    return out


def build_program(N, B, TA, TB, TA2, TB2, nblk, shard, split):
    """Build the SPMD Bass program. T*: per-block tile counts (shared)."""
    Fin, D1, H1, C1, D2 = 128, 256, 4, 64, 64
    nxt = _ceil_div(N, 128)          # x tiles in node phase
    TT1 = int(sum(TA) + sum(TB))     # layer-1 edge tiles
    TT2 = int(sum(TA2) + sum(TB2))   # layer-2 edge tiles
    TT = TT1 + TT2
    NI16 = TT * 8                    # idx16 columns

    nc = bacc.Bacc("TRN2", target_bir_lowering=False, debug=False,
                   num_devices=NCORES)

    # ---- I/O ----
    xin = nc.dram_tensor("x", [N, Fin], F32, kind="ExternalInput")
    w1t = nc.dram_tensor("w1text", [Fin, D1 + 8], F32, kind="ExternalInput")
    w2ta = nc.dram_tensor("w2texta", [128, PK_COLS], F32, kind="ExternalInput")
    w2tb = nc.dram_tensor("w2textb", [128, PK_COLS], F32, kind="ExternalInput")
    b1b = nc.dram_tensor("b1b", [128, D1], F32, kind="ExternalInput")
    b2b = nc.dram_tensor("b2b", [128, D2], F32, kind="ExternalInput")
    ident = nc.dram_tensor("ident", [128, 128], F32, kind="ExternalInput")
    iotar = nc.dram_tensor("iotar", [128, 128], F32, kind="ExternalInput")
    idx16 = nc.dram_tensor("idx16", [128, NI16], I16, kind="ExternalInput")
    dstrel = nc.dram_tensor("dstrel", [128, TT], F32, kind="ExternalInput")
    batchrel = nc.dram_tensor("batchrel", [128, nblk], F32, kind="ExternalInput")
    gfT = nc.dram_tensor("gfT", [32, B], F32, kind="ExternalInput")
    gw1T = nc.dram_tensor("gw1T", [32, D2], F32, kind="ExternalInput")
    gb1c = nc.dram_tensor("gb1c", [D2, 1], F32, kind="ExternalInput")
    gw2T = nc.dram_tensor("gw2T", [D2, D2], F32, kind="ExternalInput")
    gb2c = nc.dram_tensor("gb2c", [D2, 1], F32, kind="ExternalInput")
    fwT = nc.dram_tensor("fwT", [2 * D2, 16], F32, kind="ExternalInput")
    fbb = nc.dram_tensor("fbb", [B, 16], F32, kind="ExternalInput")
    out = nc.dram_tensor("out", [B, 16], F32, kind="ExternalOutput")

    # ---- internal DRAM ----
    h1a = nc.dram_tensor("h1a", [N, T1_COLS], F32)        # per-core full table
    table2 = nc.dram_tensor("table2", [N, T2_COLS], F32)  # gather table L2
    shp = nblk * 128                                  # padded shard rows
    agin = nc.dram_tensor("agin", [shp, PK_COLS], F32)
    agout = nc.dram_tensor("agout", [NCORES * shp, PK_COLS], F32)
    prin = nc.dram_tensor("prin", [B, D2 + 1], F32)       # pool partial in
    prout = nc.dram_tensor("prout", [B, D2 + 1], F32)     # pool reduced

    with tile.TileContext(nc) as tc:
        with (
            tc.tile_pool(name="const", bufs=1) as cpool,
            tc.tile_pool(name="sb", bufs=3) as sb,
            tc.tile_pool(name="gp", bufs=3) as gpool,
            tc.tile_pool(name="wk", bufs=3) as wk,
            tc.tile_pool(name="ps", bufs=2, space="PSUM") as ps,
            tc.tile_pool(name="pss", bufs=2, space="PSUM") as pss,
            tc.tile_pool(name="pssm", bufs=1, space="PSUM") as pssm,
            tc.tile_pool(name="psp", bufs=1, space="PSUM") as psp,
            tc.tile_pool(name="dram", bufs=1, space="DRAM") as dpool,
        ):
            nc.gpsimd.load_library(library_config.mlp)

            # ---- resident constants ----
            w1t_sb = cpool.tile([Fin, D1 + 8], F32)
            nc.sync.dma_start(out=w1t_sb[:], in_=w1t[:])
            w2ta_sb = cpool.tile([128, PK_COLS], F32)
            nc.sync.dma_start(out=w2ta_sb[:], in_=w2ta[:])
            w2tb_sb = cpool.tile([128, PK_COLS], F32)
            nc.sync.dma_start(out=w2tb_sb[:], in_=w2tb[:])
            b1b_sb = cpool.tile([128, D1], F32)
            nc.sync.dma_start(out=b1b_sb[:], in_=b1b[:])
            b2b_sb = cpool.tile([128, D2], F32)
            nc.sync.dma_start(out=b2b_sb[:], in_=b2b[:])
            id_sb = cpool.tile([128, 128], F32)
            nc.sync.dma_start(out=id_sb[:], in_=ident[:])
            iot_sb = cpool.tile([128, 128], F32)
            nc.sync.dma_start(out=iot_sb[:], in_=iotar[:])
            id_bf = cpool.tile([128, 128], BF16)
            nc.vector.tensor_copy(out=id_bf[:], in_=id_sb[:])
            idx_sb = cpool.tile([128, NI16], I16)
            nc.sync.dma_start(out=idx_sb[:], in_=idx16[:])
            dst_sb = cpool.tile([128, TT], F32)
            nc.sync.dma_start(out=dst_sb[:], in_=dstrel[:])
            bat_sb = cpool.tile([128, nblk], F32)
            nc.sync.dma_start(out=bat_sb[:], in_=batchrel[:])

            # ================= P1: node phase, build h1a =================
            for it in range(nxt):
                r0 = it * 128
                rows = min(128, N - r0)
                xt = sb.tile([128, Fin], F32, tag="xt")
                nc.sync.dma_start(out=xt[:rows], in_=xin[r0:r0 + rows, :])
                xT_ps = pss.tile([128, 128], F32, tag="t128")
                nc.tensor.transpose(out=xT_ps[:, :rows], in_=xt[:rows],
                                    identity=id_sb[:rows, :rows])
                xT = sb.tile([128, 128], F32, tag="xT")
                nc.vector.tensor_copy(out=xT[:, :rows], in_=xT_ps[:, :rows])
                h1_ps = ps.tile([128, D1 + 8], F32, tag="eacc")
                nc.tensor.matmul(out=h1_ps[:rows], lhsT=xT[:, :rows],
                                 rhs=w1t_sb[:], start=True, stop=True)
                asm = sb.tile([128, T1_COLS], F32, tag="asm")
                nc.vector.memset(asm[:, T1_AD + 4:], 0.0)
                nc.vector.tensor_copy(out=asm[:rows, :D1 + 8], in_=h1_ps[:rows])
                nc.sync.dma_start(out=h1a[r0:r0 + rows, :], in_=asm[:rows])

            # ================= P2/P3: layer-1 edge phase =================
            tile_idx = 0        # global edge-tile cursor
            icol = 0            # idx16 column cursor
            for b in range(nblk):
                brows = min(128, shard - b * 128)
                a1d_blk = wk.tile([128, 4], F32, tag="a1d")
                nc.sync.dma_start(out=a1d_blk[:],
                                  in_=h1a[b * 128:(b + 1) * 128,
                                          T1_AD:T1_AD + 4])
                a1d_bf = wk.tile([128, 4], BF16, tag="a1dbf")
                nc.vector.tensor_copy(out=a1d_bf[:], in_=a1d_blk[:])
                out_ps = ps.tile([128, D1 + 4], F32, tag="eacc")
                plans = ([("A", n) for n in _call_sizes(TA[b])]
                         + [("B", n) for n in _call_sizes(TB[b])])
                first = True
                total_calls = len(plans)
                for ci, (half, nt) in enumerate(plans):
                    nidx = nt * 128
                    G = gpool.tile([128, CALL_TILES * T1_COLS], F32, tag="G1")
                    src_view = h1a[0:split] if half == "A" else h1a[split:N]
                    nc.gpsimd.dma_gather(
                        G[:].rearrange("p (t c) -> p t c", c=T1_COLS)[:, :nt],
                        src_view[:, :],
                        idx_sb[:, icol:icol + nidx // 16],
                        nidx, nidx, T1_COLS,
                    )
                    icol += nidx // 16
                    # selection matrices + a_d expansion + weights, batched
                    ST = wk.tile([128, CALL_TILES * 128], BF16, tag="ST")
                    nc.vector.tensor_tensor(
                        out=ST[:].rearrange("p (t s) -> p t s", s=128)[:, :nt],
                        in0=dst_sb[:, tile_idx:tile_idx + nt, None]
                            .to_broadcast([128, nt, 128]),
                        in1=iot_sb[:, None, :].to_broadcast([128, nt, 128]),
                        op=mybir.AluOpType.is_equal,
                    )
                    ad_ps = pssm.tile([128, CALL_TILES * 4], F32, tag="sm")
                    for t in range(nt):
                        S_ps = pss.tile([128, 128], BF16, tag="Sbf")
                        nc.tensor.transpose(
                            out=S_ps[:], in_=ST[:, t * 128:(t + 1) * 128],
                            identity=id_bf[:])
                        S = wk.tile([128, 128], BF16, tag="S")
                        nc.vector.tensor_copy(out=S[:], in_=S_ps[:])
                        nc.tensor.matmul(out=ad_ps[:, t * 4:(t + 1) * 4],
                                         lhsT=S[:], rhs=a1d_bf[:, :],
                                         start=True, stop=True)
                    e4 = wk.tile([128, CALL_TILES * 4], F32, tag="e4")
                    nc.vector.tensor_add(
                        out=e4[:, :nt * 4],
                        in0=G[:].rearrange("p (t c) -> p t c", c=T1_COLS)
                            [:, :nt, T1_AS:T1_AS + 4],
                        in1=ad_ps[:].rearrange("p (t h) -> p t h", h=4)[:, :nt],
                    )
                    lr = wk.tile([128, CALL_TILES * 4], F32, tag="lr")
                    nc.vector.tensor_scalar_mul(out=lr[:, :nt * 4],
                                                in0=e4[:, :nt * 4],
                                                scalar1=NEG_SLOPE)
                    nc.vector.tensor_max(out=lr[:, :nt * 4],
                                         in0=lr[:, :nt * 4],
                                         in1=e4[:, :nt * 4])
                    w = wk.tile([128, CALL_TILES * 4], F32, tag="w")
                    nc.scalar.activation(out=w[:, :nt * 4], in_=lr[:, :nt * 4],
                                         func=mybir.ActivationFunctionType.Exp)
                    WH = wk.tile([128, CALL_TILES * (D1 + 4)], BF16, tag="WH")
                    nc.vector.tensor_mul(
                        out=WH[:].rearrange("p (t c) -> p t c", c=D1 + 4)
                            [:, :nt, :D1].rearrange("p t (h x) -> p t h x", x=C1),
                        in0=G[:].rearrange("p (t c) -> p t c", c=T1_COLS)
                            [:, :nt, :D1].rearrange("p t (h x) -> p t h x", x=C1),
                        in1=w[:].rearrange("p (t h) -> p t h", h=4)
                            [:, :nt, :, None].to_broadcast([128, nt, 4, C1]),
                    )
                    nc.vector.tensor_copy(
                        out=WH[:].rearrange("p (t c) -> p t c", c=D1 + 4)
                            [:, :nt, D1:],
                        in_=w[:].rearrange("p (t h) -> p t h", h=4)[:, :nt],
                    )
                    for t in range(nt):
                        nc.tensor.matmul(
                            out=out_ps[:],
                            lhsT=ST[:, t * 128:(t + 1) * 128],
                            rhs=WH[:, t * (D1 + 4):(t + 1) * (D1 + 4)],
                            start=first and t == 0,
                            stop=(ci == total_calls - 1) and (t == nt - 1),
                        )
                    first = False
                    tile_idx += nt

                # ---- P3: finalize block -> elu1, h2 pack, agin ----
                den = wk.tile([128, 4], F32, tag="den")
                nc.vector.tensor_scalar_max(out=den[:], in0=out_ps[:, D1:],
                                            scalar1=1e-30)
                rec = wk.tile([128, 4], F32, tag="rec")
                nc.vector.reciprocal(out=rec[:], in_=den[:])
                o1 = wk.tile([128, D1], F32, tag="o1")
                nc.vector.tensor_mul(
                    out=o1[:].rearrange("p (h x) -> p h x", x=C1),
                    in0=out_ps[:, :D1].rearrange("p (h x) -> p h x", x=C1),
                    in1=rec[:, :, None].to_broadcast([128, 4, C1]),
                )
                nc.vector.tensor_add(out=o1[:], in0=o1[:], in1=b1b_sb[:])
                # elu: out = max(x,0)-1 + exp(min(x,0))
                t1 = wk.tile([128, D1], F32, tag="t1")
                nc.vector.tensor_scalar_min(out=t1[:], in0=o1[:], scalar1=0.0)
                nc.scalar.activation(out=t1[:], in_=t1[:],
                                     func=mybir.ActivationFunctionType.Exp)
                nc.vector.tensor_scalar(out=o1[:], in0=o1[:], scalar1=0.0,
                                        scalar2=-1.0,
                                        op0=mybir.AluOpType.max,
                                        op1=mybir.AluOpType.add)
                e1 = wk.tile([128, D1], F32, tag="e1")
                nc.vector.tensor_add(out=e1[:], in0=o1[:], in1=t1[:])
                # h2 pack = elu1 @ W2Text  (contraction over 256, 2 chunks)
                h2_ps = ps.tile([128, PK_COLS], F32, tag="eacc")
                for ch, w2x in ((0, w2ta_sb), (1, w2tb_sb)):
                    eT_ps = pss.tile([128, 128], F32, tag="t128")
                    nc.tensor.transpose(out=eT_ps[:],
                                        in_=e1[:, ch * 128:(ch + 1) * 128],
                                        identity=id_sb[:])
                    eT = wk.tile([128, 128], F32, tag="eT")
                    nc.vector.tensor_copy(out=eT[:], in_=eT_ps[:])
                    nc.tensor.matmul(out=h2_ps[:], lhsT=eT[:], rhs=w2x[:],
                                     start=(ch == 0), stop=(ch == 1))
                h2p = wk.tile([128, PK_COLS], F32, tag="h2p")
                nc.vector.tensor_copy(out=h2p[:], in_=h2_ps[:])
                nc.sync.dma_start(out=agin[b * 128:(b + 1) * 128, :],
                                  in_=h2p[:])

            # ================= P4: allgather + table2 =================
            nc.gpsimd.collective_compute(
                "AllGather", mybir.AluOpType.bypass,
                replica_groups=[list(range(NCORES))],
                ins=[agin.ap().opt()], outs=[agout.ap().opt()],
            )
            for c in range(NCORES):
                nc.sync.dma_start(
                    out=table2[c * shard:(c + 1) * shard, :PK_COLS],
                    in_=agout[c * shp:c * shp + shard, :])

            # ================= P5/P6: layer-2 edge phase =================
            pool_ps = psp.tile([128, D2 + 1], F32)  # pooling accumulator
            tile_idx = TT1
            icol2 = icol
            for b in range(nblk):
                brows = min(128, shard - b * 128)
                a2d_blk = wk.tile([128, 1], F32, tag="a2d")
                nc.sync.dma_start(out=a2d_blk[:],
                                  in_=agin[b * 128:(b + 1) * 128, 65:66])
                a2d_bf = wk.tile([128, 1], BF16, tag="a2dbf")
                nc.vector.tensor_copy(out=a2d_bf[:], in_=a2d_blk[:])
                out_ps = ps.tile([128, D2 + 1], F32, tag="eacc")
                plans = ([("A", n) for n in _call_sizes(TA2[b])]
                         + [("B", n) for n in _call_sizes(TB2[b])])
                first = True
                total_calls = len(plans)
                for ci, (half, nt) in enumerate(plans):
                    nidx = nt * 128
                    G = gpool.tile([128, CALL_TILES * T2_COLS], F32, tag="G2")
                    src_view = table2[0:split] if half == "A" else table2[split:N]
                    nc.gpsimd.dma_gather(
                        G[:].rearrange("p (t c) -> p t c", c=T2_COLS)[:, :nt],
                        src_view[:, :],
                        idx_sb[:, icol2:icol2 + nidx // 16],
                        nidx, nidx, T2_COLS,
                    )
                    icol2 += nidx // 16
                    ST = wk.tile([128, CALL_TILES * 128], BF16, tag="ST")
                    nc.vector.tensor_tensor(
                        out=ST[:].rearrange("p (t s) -> p t s", s=128)[:, :nt],
                        in0=dst_sb[:, tile_idx:tile_idx + nt, None]
                            .to_broadcast([128, nt, 128]),
                        in1=iot_sb[:, None, :].to_broadcast([128, nt, 128]),
                        op=mybir.AluOpType.is_equal,
                    )
                    ad_ps = pssm.tile([128, CALL_TILES], F32, tag="sm")
                    for t in range(nt):
                        S_ps = pss.tile([128, 128], BF16, tag="Sbf")
                        nc.tensor.transpose(
                            out=S_ps[:], in_=ST[:, t * 128:(t + 1) * 128],
                            identity=id_bf[:])
                        S = wk.tile([128, 128], BF16, tag="S")
                        nc.vector.tensor_copy(out=S[:], in_=S_ps[:])
                        nc.tensor.matmul(out=ad_ps[:, t:t + 1],
                                         lhsT=S[:], rhs=a2d_bf[:, :],
                                         start=True, stop=True)
                    e4 = wk.tile([128, CALL_TILES], F32, tag="e4")
                    nc.vector.tensor_add(
                        out=e4[:, :nt],
                        in0=G[:].rearrange("p (t c) -> p t c", c=T2_COLS)
                            [:, :nt, T2_AS],
                        in1=ad_ps[:, :nt],
                    )
                    lr = wk.tile([128, CALL_TILES], F32, tag="lr")
                    nc.vector.tensor_scalar_mul(out=lr[:, :nt],
                                                in0=e4[:, :nt],
                                                scalar1=NEG_SLOPE)
                    nc.vector.tensor_max(out=lr[:, :nt], in0=lr[:, :nt],
                                         in1=e4[:, :nt])
                    w = wk.tile([128, CALL_TILES], F32, tag="w")
                    nc.scalar.activation(out=w[:, :nt], in_=lr[:, :nt],
                                         func=mybir.ActivationFunctionType.Exp)
                    WH = wk.tile([128, CALL_TILES * (D2 + 1)], BF16, tag="WH")
                    nc.vector.tensor_mul(
                        out=WH[:].rearrange("p (t c) -> p t c", c=D2 + 1)
                            [:, :nt, :D2],
                        in0=G[:].rearrange("p (t c) -> p t c", c=T2_COLS)
                            [:, :nt, :D2],
                        in1=w[:, :nt, None].to_broadcast([128, nt, D2]),
                    )
                    nc.vector.tensor_copy(
                        out=WH[:].rearrange("p (t c) -> p t c", c=D2 + 1)
                            [:, :nt, D2],
                        in_=w[:, :nt],
                    )
                    for t in range(nt):
                        nc.tensor.matmul(
                            out=out_ps[:],
                            lhsT=ST[:, t * 128:(t + 1) * 128],
                            rhs=WH[:, t * (D2 + 1):(t + 1) * (D2 + 1)],
                            start=first and t == 0,
                            stop=(ci == total_calls - 1) and (t == nt - 1),
                        )
                    first = False
                    tile_idx += nt

                # ---- P6: finalize block -> elu2, pooling ----
                den = wk.tile([128, 1], F32, tag="den2")
                nc.vector.tensor_scalar_max(out=den[:], in0=out_ps[:, D2:],
                                            scalar1=1e-30)
                rec = wk.tile([128, 1], F32, tag="rec2")
                nc.vector.reciprocal(out=rec[:], in_=den[:])
                o2 = wk.tile([128, D2 + 1], F32, tag="o2")
                nc.vector.tensor_mul(out=o2[:, :D2], in0=out_ps[:, :D2],
                                     in1=rec[:, :1].to_broadcast([128, D2]))
                nc.vector.tensor_add(out=o2[:, :D2], in0=o2[:, :D2],
                                     in1=b2b_sb[:])
                t1 = wk.tile([128, D2], F32, tag="t12")
                nc.vector.tensor_scalar_min(out=t1[:], in0=o2[:, :D2],
                                            scalar1=0.0)
                nc.scalar.activation(out=t1[:], in_=t1[:],
                                     func=mybir.ActivationFunctionType.Exp)
                nc.vector.tensor_scalar(out=o2[:, :D2], in0=o2[:, :D2],
                                        scalar1=0.0, scalar2=-1.0,
                                        op0=mybir.AluOpType.max,
                                        op1=mybir.AluOpType.add)
                nc.vector.tensor_add(out=o2[:, :D2], in0=o2[:, :D2], in1=t1[:])
                nc.vector.memset(o2[:, D2:], 1.0)
                pST = wk.tile([128, 128], F32, tag="pST")
                nc.vector.tensor_tensor(
                    out=pST[:],
                    in0=bat_sb[:, b:b + 1].to_broadcast([128, 128]),
                    in1=iot_sb[:],
                    op=mybir.AluOpType.is_equal,
                )
                nc.tensor.matmul(out=pool_ps[:], lhsT=pST[:], rhs=o2[:],
                                 start=(b == 0), stop=(b == nblk - 1))

            # ================= P7: pool allreduce + tail =================
            pp = wk.tile([B, D2 + 1], F32, tag="pp")
            nc.vector.tensor_copy(out=pp[:], in_=pool_ps[:B])
            nc.sync.dma_start(out=prin[:, :], in_=pp[:])
            nc.gpsimd.collective_compute(
                "AllReduce", mybir.AluOpType.add,
                replica_groups=[list(range(NCORES))],
                ins=[prin.ap().opt()], outs=[prout.ap().opt()],
            )
            pr = wk.tile([B, D2 + 1], F32, tag="pr")
            nc.sync.dma_start(out=pr[:], in_=prout[:, :])
            cnt = wk.tile([B, 1], F32, tag="cnt")
            nc.vector.tensor_scalar_max(out=cnt[:], in0=pr[:, D2:],
                                        scalar1=1.0)
            rcnt = wk.tile([B, 1], F32, tag="rcnt")
            nc.vector.reciprocal(out=rcnt[:], in_=cnt[:])
            pooled = wk.tile([B, D2], F32, tag="pooled")
            nc.vector.tensor_mul(out=pooled[:], in0=pr[:, :D2],
                                 in1=rcnt[:, :1].to_broadcast([B, D2]))
            zT = wk.tile([128, B], F32, tag="zT")
            pT_ps = pssm.tile([128, B], F32, tag="sm")
            nc.tensor.transpose(out=pT_ps[:D2, :B], in_=pooled[:],
                                identity=id_sb[:B, :B])
            nc.vector.tensor_copy(out=zT[:D2], in_=pT_ps[:D2, :B])
            # g-MLP (transposed layout: [feat, graph])
            gf_sb = wk.tile([32, B], F32, tag="gf")
            nc.sync.dma_start(out=gf_sb[:], in_=gfT[:])
            gw1_sb = wk.tile([32, D2], F32, tag="gw1")
            nc.sync.dma_start(out=gw1_sb[:], in_=gw1T[:])
            gb1_sb = wk.tile([D2, 1], F32, tag="gb1")
            nc.sync.dma_start(out=gb1_sb[:], in_=gb1c[:])
            gw2_sb = wk.tile([D2, D2], F32, tag="gw2")
            nc.sync.dma_start(out=gw2_sb[:], in_=gw2T[:])
            gb2_sb = wk.tile([D2, 1], F32, tag="gb2")
            nc.sync.dma_start(out=gb2_sb[:], in_=gb2c[:])
            fw_sb = wk.tile([2 * D2, 16], F32, tag="fw")
            nc.sync.dma_start(out=fw_sb[:], in_=fwT[:])
            fb_sb = wk.tile([B, 16], F32, tag="fb")
            nc.sync.dma_start(out=fb_sb[:], in_=fbb[:])

            g1_ps = pssm.tile([D2, B], F32, tag="sm")
            nc.tensor.matmul(out=g1_ps[:], lhsT=gw1_sb[:], rhs=gf_sb[:],
                             start=True, stop=True)
            g1 = wk.tile([D2, B], F32, tag="g1")
            nc.scalar.activation(out=g1[:], in_=g1_ps[:],
                                 func=mybir.ActivationFunctionType.Relu,
                                 bias=gb1_sb[:, :1])
            g2_ps = pssm.tile([D2, B], F32, tag="sm")
            nc.tensor.matmul(out=g2_ps[:], lhsT=gw2_sb[:], rhs=g1[:],
                             start=True, stop=True)
            nc.scalar.activation(out=zT[D2:2 * D2], in_=g2_ps[:],
                                 func=mybir.ActivationFunctionType.Relu,
                                 bias=gb2_sb[:, :1])
            fin_ps = pssm.tile([B, 16], F32, tag="sm")
            nc.tensor.matmul(out=fin_ps[:], lhsT=zT[:, :B], rhs=fw_sb[:],
                             start=True, stop=True)
            fin = wk.tile([B, 16], F32, tag="fin")
            nc.vector.tensor_add(out=fin[:], in0=fin_ps[:], in1=fb_sb[:])
            nc.sync.dma_start(out=out[:, :], in_=fin[:])

    nc.compile()
    return nc


def _wrap_idx(idx_flat):
    """dma_gather idx layout: [128, n/16] int16, replicated per 16-row group."""
    n = len(idx_flat)
    a = np.empty((128, n // 16), dtype=np.int16)
    blk = idx_flat.reshape(n // 16, 16).T
    for g in range(8):
        a[g * 16:(g + 1) * 16, :] = blk
    return a


def host_prep(inputs, N, B, ncores=NCORES):
    """Sort/shard edges, build per-core tables. Returns (meta, in_maps)."""
    x = np.asarray(inputs["x"], np.float32)
    ei = np.asarray(inputs["edge_index"], np.int64)
    batch = np.asarray(inputs["batch"], np.int64)
    shard = N // ncores
    nblk = _ceil_div(shard, 128)
    split = ((N // 2 + 127) // 128) * 128
    assert split < 32768 or N <= 32768, "split must fit int16"
    if N <= 32768:
        split = N  # everything in half A

    loop = np.arange(N, dtype=np.int64)
    src = np.concatenate([ei[0], loop])
    dst = np.concatenate([ei[1], loop])
    order = np.argsort(dst, kind="stable")
    src, dst = src[order], dst[order]

    # per-core edge lists; per-block A/B sublists for both layers
    # layer 1 gathers from the per-core PERMUTED h1a table (positions);
    # layer 2 gathers from the GLOBAL table2 (global source ids).
    per_core = []
    nA1 = np.zeros((ncores, nblk), np.int64)
    nB1 = np.zeros((ncores, nblk), np.int64)
    nA2 = np.zeros((ncores, nblk), np.int64)
    nB2 = np.zeros((ncores, nblk), np.int64)
    for c in range(ncores):
        own0, own1 = c * shard, (c + 1) * shard
        sel = (dst >= own0) & (dst < own1)
        s_c, d_c = src[sel], dst[sel] - own0
        pos = np.where((s_c >= own0) & (s_c < own1), s_c - own0,
                       np.where(s_c < own0, s_c, s_c - shard) + shard)
        blocks = []
        for b in range(nblk):
            lo = np.searchsorted(d_c, b * 128)
            hi = np.searchsorted(d_c, min((b + 1) * 128, shard))
            pb, gb, db = pos[lo:hi], s_c[lo:hi], d_c[lo:hi] - b * 128
            mA = pb < split
            L1A = (pb[mA], db[mA])
            L1B = (pb[~mA] - split, db[~mA])
            gA = gb < split
            L2A = (gb[gA], db[gA])
            L2B = (gb[~gA] - split, db[~gA])
            blocks.append(((L1A, L1B), (L2A, L2B)))
            nA1[c, b], nB1[c, b] = len(L1A[0]), len(L1B[0])
            nA2[c, b], nB2[c, b] = len(L2A[0]), len(L2B[0])
        per_core.append(blocks)

    TA = [int(_ceil_div(int(nA1[:, b].max()), 128)) for b in range(nblk)]
    TB = [int(_ceil_div(int(nB1[:, b].max()), 128)) for b in range(nblk)]
    TA2 = [int(_ceil_div(int(nA2[:, b].max()), 128)) for b in range(nblk)]
    TB2 = [int(_ceil_div(int(nB2[:, b].max()), 128)) for b in range(nblk)]
    TT = sum(TA) + sum(TB) + sum(TA2) + sum(TB2)

    # per-core packed arrays
    in_cores = []
    for c in range(ncores):
        idx_cols, dst_cols = [], []
        for layer in (0, 1):
            Tmax = (TA, TB) if layer == 0 else (TA2, TB2)
            for b in range(nblk):
                (A, Bm) = per_core[c][b][layer]
                for (T, (p_arr, d_arr)) in ((Tmax[0][b], A), (Tmax[1][b], Bm)):
                    if T == 0:
                        continue
                    n = T * 128
                    pi = np.zeros(n, np.int16)
                    pi[:len(p_arr)] = p_arr.astype(np.int16)
                    di = np.full(n, 200.0, np.float32)
                    di[:len(d_arr)] = d_arr.astype(np.float32)
                    base = 0
                    for ct in _call_sizes(T):
                        idx_cols.append(_wrap_idx(pi[base:base + ct * 128]))
                        base += ct * 128
                    dst_cols.append(di.reshape(T, 128).T)
        idx16 = np.concatenate(idx_cols, axis=1)
        dstrel = np.concatenate(dst_cols, axis=1)
        own0 = c * shard
        bat = np.full((128, nblk), 200.0, np.float32)
        for b in range(nblk):
            rows = min(128, shard - b * 128)
            bat[:rows, b] = batch[own0 + b * 128: own0 + b * 128 + rows]
        perm = np.concatenate([
            np.arange(own0, own0 + shard),
            np.arange(0, own0),
            np.arange(own0 + shard, N),
        ])
        in_cores.append({
            "idx16": idx16, "dstrel": dstrel, "batchrel": bat,
            "x": np.ascontiguousarray(x[perm]),
        })

    # shared weights
    W1 = np.asarray(inputs["W1"], np.float32)       # [256, 128]
    a1s = np.asarray(inputs["a1s"], np.float32)     # [4, 64]
    a1d = np.asarray(inputs["a1d"], np.float32)
    b1 = np.asarray(inputs["b1"], np.float32)
    W2 = np.asarray(inputs["W2"], np.float32)       # [64, 256]
    a2s = np.asarray(inputs["a2s"], np.float32)     # [1, 64]
    a2d = np.asarray(inputs["a2d"], np.float32)
    b2 = np.asarray(inputs["b2"], np.float32)
    H1, C1, Fin = a1s.shape[0], a1s.shape[1], W1.shape[1]
    W1r = W1.reshape(H1, C1, Fin)
    Vs = np.einsum("hcf,hc->fh", W1r, a1s)
    Vd = np.einsum("hcf,hc->fh", W1r, a1d)
    w1text = np.concatenate([W1.T, Vs, Vd], axis=1).astype(np.float32)
    V2s = (a2s[0] @ W2)[:, None]
    V2d = (a2d[0] @ W2)[:, None]
    w2text = np.concatenate([W2.T, V2s, V2d], axis=1).astype(np.float32)
    shared = {
        "w1text": w1text,
        "w2texta": w2text[:128], "w2textb": w2text[128:256],
        "b1b": np.tile(b1, (128, 1)).astype(np.float32),
        "b2b": np.tile(b2, (128, 1)).astype(np.float32),
        "ident": np.eye(128, dtype=np.float32),
        "iotar": np.tile(np.arange(128, dtype=np.float32), (128, 1)),
        "gfT": np.ascontiguousarray(
            np.asarray(inputs["global_feats"], np.float32).T),
        "gw1T": np.ascontiguousarray(np.asarray(inputs["gw1"], np.float32).T),
        "gb1c": np.asarray(inputs["gb1"], np.float32)[:, None],
        "gw2T": np.ascontiguousarray(np.asarray(inputs["gw2"], np.float32).T),
        "gb2c": np.asarray(inputs["gb2"], np.float32)[:, None],
        "fwT": np.ascontiguousarray(np.asarray(inputs["fw"], np.float32).T),
        "fbb": np.tile(np.asarray(inputs["fb"], np.float32), (B, 1)),
    }
    in_maps = [{**shared, **pc} for pc in in_cores]
    meta = dict(N=N, B=B, TA=TA, TB=TB, TA2=TA2, TB2=TB2, nblk=nblk,
                shard=shard, split=split, TT=TT)
    return meta, in_maps


_PROGRAM_CACHE = {}


def run(inputs, N, B, trace=False):
    meta, in_maps = host_prep(inputs, N, B)
    key = (N, B, tuple(meta["TA"]), tuple(meta["TB"]),
           tuple(meta["TA2"]), tuple(meta["TB2"]))
    nc = _PROGRAM_CACHE.get(key)
    if nc is None:
        nc = build_program(N, B, meta["TA"], meta["TB"], meta["TA2"],
                           meta["TB2"], meta["nblk"], meta["shard"],
                           meta["split"])
        _PROGRAM_CACHE[key] = nc
    res = bass_utils.run_bass_kernel_spmd(
        nc, in_maps, core_ids=list(range(NCORES)), trace=trace)
    return np.asarray(res.results[0]["out"]), res


def kernel(**inputs) -> np.ndarray:
    N = int(np.asarray(inputs["x"]).shape[0])
    B = int(np.asarray(inputs["global_feats"]).shape[0])
    out, _ = run(inputs, N, B)
    return out



# revision 5
# speedup vs baseline: 1.1424x; 1.1424x over previous
"""GAT (2-layer) + global pooling Trainium2 kernel, 8-way SPMD. v2.

Sharding: destination-node sharding for both GAT layers. Each core owns a
contiguous range of 6250 destination nodes and all edges incident to them.
Per-core node permutation puts owned nodes at table rows [0, 6250) so the
program is identical across cores (pure SPMD, data-only divergence).

v2 changes vs baseline:
- All gather tables bf16: L1 rows 768B ([h1(256)|as(4)|ad(4)|pad] bf16),
  L2 rows 256B gathered directly from the allgather output (table2 removed).
- Node phase takes pre-transposed bf16 x from host (no device transposes),
  bf16 matmul.
- Per-tile transpose+copy of the selection matrix replaced by a second
  direct IS_EQ build of S (dst-on-partition layout) fed by a DMA
  partition-broadcast of the per-edge dst row; ad matmuls take lhsT slices
  of S.
- Leaky-relu fused to one DVE op; exp emits bf16; softmax division moved to
  the scalar engine (per-head Copy with per-partition scale); elu computed
  as y=elu+1 (max(x,0)+exp(min(x,0))) with the -1 folded into the next
  layer's weights (corr row) / the final bias (host-adjusted fb).
- CALL_TILES 16 (fewer dma_gather fixed overheads).
"""
import sys
import numpy as np

for _p in ("/opt/trn_rl_repo", "/root/.axon_site"):
    if _p not in sys.path:
        sys.path.insert(0, _p)

import ml_dtypes
import concourse.bass as bass
import concourse.bacc as bacc
import concourse.tile as tile
import concourse.mybir as mybir
from concourse import bass_utils, library_config

F32 = mybir.dt.float32
BF16 = mybir.dt.bfloat16
I16 = mybir.dt.int16

NCORES = 8
NEG_SLOPE = 0.2
CALL_TILES = 8          # max tiles (of 128 edges) per dma_gather call

# layer-1 table row (bf16): [h1 (256) | a1s (4) | a1d (4) | pad] -> 384 cols
T1_H, T1_AS, T1_AD, T1_COLS = 256, 256, 260, 384
# layer-2 row = allgather pack row (bf16): [h2 (64) | a2s | a2d | pad] -> 128
T2_H, T2_AS, T2_AD, T2_COLS = 64, 64, 65, 128


def _ceil_div(a, b):
    return -(-a // b)


def _call_sizes(ntiles):
    out = []
    while ntiles > 0:
        c = min(ntiles, CALL_TILES)
        out.append(c)
        ntiles -= c
    return out


def build_program(N, B, TA, TB, TA2, TB2, nblk, shard, split):
    """Build the SPMD Bass program. T*: per-block tile counts (shared)."""
    Fin, D1, H1, C1, D2 = 128, 256, 4, 64, 64
    nxt = _ceil_div(N, 128)          # x tiles in node phase
    TT1 = int(sum(TA) + sum(TB))     # layer-1 edge tiles
    TT2 = int(sum(TA2) + sum(TB2))   # layer-2 edge tiles
    TT = TT1 + TT2
    NI16 = TT * 8                    # idx16 columns
    shp = nblk * 128                 # padded shard rows
    asplit = (NCORES // 2) * shp     # agout A/B row split

    nc = bacc.Bacc("TRN2", target_bir_lowering=False, debug=False,
                   num_devices=NCORES)

    # ---- I/O ----
    xT = nc.dram_tensor("xT", [Fin, N], BF16, kind="ExternalInput")
    w1t = nc.dram_tensor("w1text", [Fin, D1 + 8], BF16, kind="ExternalInput")
    w2ta = nc.dram_tensor("w2texta", [128, 66], BF16, kind="ExternalInput")
    w2tb = nc.dram_tensor("w2textb", [128, 66], BF16, kind="ExternalInput")
    corrb = nc.dram_tensor("corrb", [128, 66], F32, kind="ExternalInput")
    b1b = nc.dram_tensor("b1b", [128, D1], F32, kind="ExternalInput")
    b2b = nc.dram_tensor("b2b", [128, D2], F32, kind="ExternalInput")
    ident = nc.dram_tensor("ident", [128, 128], F32, kind="ExternalInput")
    iotar = nc.dram_tensor("iotar", [128, 128], BF16, kind="ExternalInput")
    iotac = nc.dram_tensor("iotac", [128, 1], BF16, kind="ExternalInput")
    idx16 = nc.dram_tensor("idx16", [128, NI16], I16, kind="ExternalInput")
    dstrel = nc.dram_tensor("dstrel", [128, TT], BF16, kind="ExternalInput")
    dstT = nc.dram_tensor("dstT", [1, TT * 128], BF16, kind="ExternalInput")
    batchrel = nc.dram_tensor("batchrel", [128, nblk], BF16,
                              kind="ExternalInput")
    gfT = nc.dram_tensor("gfT", [32, B], F32, kind="ExternalInput")
    gw1T = nc.dram_tensor("gw1T", [32, D2], F32, kind="ExternalInput")
    gb1c = nc.dram_tensor("gb1c", [D2, 1], F32, kind="ExternalInput")
    gw2T = nc.dram_tensor("gw2T", [D2, D2], F32, kind="ExternalInput")
    gb2c = nc.dram_tensor("gb2c", [D2, 1], F32, kind="ExternalInput")
    fwT = nc.dram_tensor("fwT", [2 * D2, 16], F32, kind="ExternalInput")
    fbb = nc.dram_tensor("fbb", [B, 16], F32, kind="ExternalInput")
    out = nc.dram_tensor("out", [B, 16], F32, kind="ExternalOutput")

    # ---- internal DRAM ----
    h1a = nc.dram_tensor("h1a", [N, T1_COLS], BF16)       # per-core table L1
    agin = nc.dram_tensor("agin", [shp, T2_COLS], BF16)
    agout = nc.dram_tensor("agout", [NCORES * shp, T2_COLS], BF16)
    prin = nc.dram_tensor("prin", [B, D2 + 1], F32)       # pool partial in
    prout = nc.dram_tensor("prout", [B, D2 + 1], F32)     # pool reduced

    with tile.TileContext(nc) as tc:
        with (
            tc.tile_pool(name="const", bufs=1) as cpool,
            tc.tile_pool(name="xp", bufs=3) as xpool,
            tc.tile_pool(name="gp", bufs=3) as gpool,
            tc.tile_pool(name="bp", bufs=3) as bpool,
            tc.tile_pool(name="stp", bufs=3) as stpool,
            tc.tile_pool(name="sp", bufs=3) as spool,
            tc.tile_pool(name="whp", bufs=3) as whpool,
            tc.tile_pool(name="wk", bufs=3) as wk,
            tc.tile_pool(name="agp", bufs=3) as agpool,
            tc.tile_pool(name="psnh", bufs=2, space="PSUM") as psnh,
            tc.tile_pool(name="pse", bufs=2, space="PSUM") as pse,
            tc.tile_pool(name="psad", bufs=2, space="PSUM") as psad,
            tc.tile_pool(name="psT", bufs=1, space="PSUM") as psT,
            tc.tile_pool(name="psp", bufs=1, space="PSUM") as psp,
        ):
            nc.gpsimd.load_library(library_config.mlp)

            # ---- resident constants ----
            w1t_sb = cpool.tile([Fin, D1 + 8], BF16)
            nc.sync.dma_start(out=w1t_sb[:], in_=w1t[:])
            w2ta_sb = cpool.tile([128, 66], BF16)
            nc.sync.dma_start(out=w2ta_sb[:], in_=w2ta[:])
            w2tb_sb = cpool.tile([128, 66], BF16)
            nc.sync.dma_start(out=w2tb_sb[:], in_=w2tb[:])
            corr_sb = cpool.tile([128, 66], F32)
            nc.sync.dma_start(out=corr_sb[:], in_=corrb[:])
            b1b_sb = cpool.tile([128, D1], F32)
            nc.sync.dma_start(out=b1b_sb[:], in_=b1b[:])
            b2b_sb = cpool.tile([128, D2], F32)
            nc.sync.dma_start(out=b2b_sb[:], in_=b2b[:])
            id_sb = cpool.tile([128, 128], F32)
            nc.sync.dma_start(out=id_sb[:], in_=ident[:])
            id_bf = cpool.tile([128, 128], BF16)
            nc.vector.tensor_copy(out=id_bf[:], in_=id_sb[:])
            iot_sb = cpool.tile([128, 128], BF16)
            nc.sync.dma_start(out=iot_sb[:], in_=iotar[:])
            ioc_sb = cpool.tile([128, 1], BF16)
            nc.sync.dma_start(out=ioc_sb[:], in_=iotac[:])
            idx_sb = cpool.tile([128, NI16], I16)
            nc.sync.dma_start(out=idx_sb[:], in_=idx16[:])
            dst_sb = cpool.tile([128, TT], BF16)
            nc.sync.dma_start(out=dst_sb[:], in_=dstrel[:])
            bat_sb = cpool.tile([128, nblk], BF16)
            nc.sync.dma_start(out=bat_sb[:], in_=batchrel[:])
            a1dres = cpool.tile([128, nblk * H1], BF16)   # own-shard a1d
            a2dres = cpool.tile([128, nblk], BF16)        # own-shard a2d

            # ================= P1: node phase, build h1a =================
            for it in range(nxt):
                r0 = it * 128
                rows = min(128, N - r0)
                xt = xpool.tile([Fin, 128], BF16, tag="xt")
                nc.sync.dma_start(out=xt[:, :rows], in_=xT[:, r0:r0 + rows])
                h1_ps = psnh.tile([128, D1 + 8], F32, tag="nh")
                nc.tensor.matmul(out=h1_ps[:rows], lhsT=xt[:, :rows],
                                 rhs=w1t_sb[:], start=True, stop=True)
                asm = agpool.tile([128, D1 + 8], BF16, tag="asm")
                nc.vector.tensor_copy(out=asm[:rows], in_=h1_ps[:rows])
                if it < nblk:
                    nc.vector.tensor_copy(
                        out=a1dres[:, it * H1:(it + 1) * H1],
                        in_=h1_ps[:, T1_AD:T1_AD + H1])
                nc.sync.dma_start(out=h1a[r0:r0 + rows, :D1 + 8],
                                  in_=asm[:rows])

            # ================= P2/P3: layer-1 edge phase =================
            tile_idx = 0        # global edge-tile cursor
            icol = 0            # idx16 column cursor
            for b in range(nblk):
                out_ps = pse.tile([128, D1 + H1], F32, tag="eacc")
                plans = ([("A", n) for n in _call_sizes(TA[b])]
                         + [("B", n) for n in _call_sizes(TB[b])])
                first = True
                total_calls = len(plans)
                for ci, (half, nt) in enumerate(plans):
                    nidx = nt * 128
                    G = gpool.tile([128, CALL_TILES * T1_COLS], BF16, tag="G")
                    src_view = h1a[0:split] if half == "A" else h1a[split:N]
                    nc.gpsimd.dma_gather(
                        G[:].rearrange("p (t c) -> p t c", c=T1_COLS)[:, :nt],
                        src_view[:, :],
                        idx_sb[:, icol:icol + nidx // 16],
                        nidx, nidx, T1_COLS,
                    )
                    icol += nidx // 16
                    # selection matrices (both layouts, no transposes)
                    ST = stpool.tile([128, CALL_TILES * 128], BF16, tag="ST")
                    nc.vector.tensor_tensor(
                        out=ST[:].rearrange("p (t s) -> p t s", s=128)[:, :nt],
                        in0=dst_sb[:, tile_idx:tile_idx + nt, None]
                            .to_broadcast([128, nt, 128]),
                        in1=iot_sb[:, None, :].to_broadcast([128, nt, 128]),
                        op=mybir.AluOpType.is_equal,
                    )
                    dbc = bpool.tile([128, CALL_TILES * 128], BF16, tag="dbc")
                    nc.sync.dma_start(
                        out=dbc[:, :nidx],
                        in_=dstT[0:1, tile_idx * 128:tile_idx * 128 + nidx]
                            .to_broadcast([128, nidx]))
                    S = spool.tile([128, CALL_TILES * 128], BF16, tag="S")
                    nc.vector.tensor_tensor(
                        out=S[:, :nidx],
                        in0=ioc_sb[:, 0, None].to_broadcast([128, nidx]),
                        in1=dbc[:, :nidx],
                        op=mybir.AluOpType.is_equal,
                    )
                    ad_ps = psad.tile([128, CALL_TILES * H1], F32, tag="sm")
                    for t in range(nt):
                        nc.tensor.matmul(
                            out=ad_ps[:, t * H1:(t + 1) * H1],
                            lhsT=S[:, t * 128:(t + 1) * 128],
                            rhs=a1dres[:, b * H1:(b + 1) * H1],
                            start=True, stop=True)
                    e4 = wk.tile([128, CALL_TILES * H1], F32, tag="e4")
                    nc.vector.tensor_add(
                        out=e4[:, :nt * H1],
                        in0=G[:].rearrange("p (t c) -> p t c", c=T1_COLS)
                            [:, :nt, T1_AS:T1_AS + H1],
                        in1=ad_ps[:].rearrange("p (t h) -> p t h", h=H1)
                            [:, :nt],
                    )
                    lr = wk.tile([128, CALL_TILES * H1], F32, tag="lr")
                    nc.vector.scalar_tensor_tensor(
                        out=lr[:, :nt * H1], in0=e4[:, :nt * H1],
                        scalar=NEG_SLOPE, in1=e4[:, :nt * H1],
                        op0=mybir.AluOpType.mult, op1=mybir.AluOpType.max)
                    w = wk.tile([128, CALL_TILES * H1], BF16, tag="w")
                    nc.scalar.activation(out=w[:, :nt * H1],
                                         in_=lr[:, :nt * H1],
                                         func=mybir.ActivationFunctionType.Exp)
                    WH = whpool.tile([128, CALL_TILES * (D1 + H1)], BF16,
                                     tag="WH")
                    nc.vector.tensor_mul(
                        out=WH[:].rearrange("p (t c) -> p t c", c=D1 + H1)
                            [:, :nt, :D1]
                            .rearrange("p t (h x) -> p t h x", x=C1),
                        in0=G[:].rearrange("p (t c) -> p t c", c=T1_COLS)
                            [:, :nt, :D1]
                            .rearrange("p t (h x) -> p t h x", x=C1),
                        in1=w[:].rearrange("p (t h) -> p t h", h=H1)
                            [:, :nt, :, None].to_broadcast([128, nt, H1, C1]),
                    )
                    nc.vector.tensor_copy(
                        out=WH[:].rearrange("p (t c) -> p t c", c=D1 + H1)
                            [:, :nt, D1:],
                        in_=w[:].rearrange("p (t h) -> p t h", h=H1)[:, :nt],
                    )
                    for t in range(nt):
                        nc.tensor.matmul(
                            out=out_ps[:],
                            lhsT=ST[:, t * 128:(t + 1) * 128],
                            rhs=WH[:, t * (D1 + H1):(t + 1) * (D1 + H1)],
                            start=first and t == 0,
                            stop=(ci == total_calls - 1) and (t == nt - 1),
                        )
                    first = False
                    tile_idx += nt

                # ---- block epilogue: softmax div, elu+1, h2 pack ----
                den = wk.tile([128, H1], F32, tag="den")
                nc.vector.tensor_scalar_max(out=den[:], in0=out_ps[:, D1:],
                                            scalar1=1e-30)
                rec = wk.tile([128, H1], F32, tag="rec")
                nc.vector.reciprocal(out=rec[:], in_=den[:])
                x1 = wk.tile([128, D1], F32, tag="x1")
                for h in range(H1):
                    nc.scalar.activation(
                        out=x1[:, h * C1:(h + 1) * C1],
                        in_=out_ps[:, h * C1:(h + 1) * C1],
                        func=mybir.ActivationFunctionType.Copy,
                        scale=rec[:, h, None])
                nc.vector.tensor_add(out=x1[:], in0=x1[:], in1=b1b_sb[:])
                # y = elu(x)+1 = max(x,0) + exp(min(x,0)); min(x,0) = -relu(-x)
                u = wk.tile([128, D1], F32, tag="u")
                nc.scalar.activation(out=u[:], in_=x1[:],
                                     func=mybir.ActivationFunctionType.Relu,
                                     scale=-1.0)
                tE = wk.tile([128, D1], F32, tag="tE")
                nc.scalar.activation(out=tE[:], in_=u[:],
                                     func=mybir.ActivationFunctionType.Exp,
                                     scale=-1.0)
                e1 = wk.tile([128, D1], BF16, tag="e1")
                nc.vector.scalar_tensor_tensor(
                    out=e1[:], in0=x1[:], scalar=0.0, in1=tE[:],
                    op0=mybir.AluOpType.max, op1=mybir.AluOpType.add)
                # h2 pack = y @ W2Text + corr  (contraction over 256, 2 chunks)
                h2_ps = psnh.tile([128, 66], F32, tag="nh")
                for ch, w2x in ((0, w2ta_sb), (1, w2tb_sb)):
                    eT_ps = psT.tile([128, 128], BF16, tag="smb")
                    nc.tensor.transpose(out=eT_ps[:],
                                        in_=e1[:, ch * 128:(ch + 1) * 128],
                                        identity=id_bf[:])
                    eT = wk.tile([128, 128], BF16, tag="eT")
                    nc.vector.tensor_copy(out=eT[:], in_=eT_ps[:])
                    nc.tensor.matmul(out=h2_ps[:], lhsT=eT[:], rhs=w2x[:],
                                     start=(ch == 0), stop=(ch == 1))
                h2p = agpool.tile([128, T2_COLS], BF16, tag="h2p")
                nc.vector.tensor_add(out=h2p[:, :66], in0=h2_ps[:],
                                     in1=corr_sb[:])
                nc.vector.tensor_copy(out=a2dres[:, b, None],
                                      in_=h2p[:, T2_AD, None])
                nc.sync.dma_start(out=agin[b * 128:(b + 1) * 128, :66],
                                  in_=h2p[:, :66])

            # ================= P4: allgather =================
            nc.gpsimd.collective_compute(
                "AllGather", mybir.AluOpType.bypass,
                replica_groups=[list(range(NCORES))],
                ins=[agin.ap().opt()], outs=[agout.ap().opt()],
            )

            # ================= P5/P6: layer-2 edge phase =================
            pool_ps = psp.tile([128, D2 + 1], F32)  # pooling accumulator
            tile_idx = TT1
            icol2 = icol
            for b in range(nblk):
                out_ps = pse.tile([128, D2 + 1], F32, tag="eacc")
                plans = ([("A", n) for n in _call_sizes(TA2[b])]
                         + [("B", n) for n in _call_sizes(TB2[b])])
                first = True
                total_calls = len(plans)
                for ci, (half, nt) in enumerate(plans):
                    nidx = nt * 128
                    G = gpool.tile([128, CALL_TILES * T1_COLS], BF16, tag="G")
                    src_view = (agout[0:asplit] if half == "A"
                                else agout[asplit:NCORES * shp])
                    nc.gpsimd.dma_gather(
                        G[:, :CALL_TILES * T2_COLS]
                            .rearrange("p (t c) -> p t c", c=T2_COLS)[:, :nt],
                        src_view[:, :],
                        idx_sb[:, icol2:icol2 + nidx // 16],
                        nidx, nidx, T2_COLS,
                    )
                    icol2 += nidx // 16
                    ST = stpool.tile([128, CALL_TILES * 128], BF16, tag="ST")
                    nc.vector.tensor_tensor(
                        out=ST[:].rearrange("p (t s) -> p t s", s=128)[:, :nt],
                        in0=dst_sb[:, tile_idx:tile_idx + nt, None]
                            .to_broadcast([128, nt, 128]),
                        in1=iot_sb[:, None, :].to_broadcast([128, nt, 128]),
                        op=mybir.AluOpType.is_equal,
                    )
                    dbc = bpool.tile([128, CALL_TILES * 128], BF16, tag="dbc")
                    nc.sync.dma_start(
                        out=dbc[:, :nidx],
                        in_=dstT[0:1, tile_idx * 128:tile_idx * 128 + nidx]
                            .to_broadcast([128, nidx]))
                    S = spool.tile([128, CALL_TILES * 128], BF16, tag="S")
                    nc.vector.tensor_tensor(
                        out=S[:, :nidx],
                        in0=ioc_sb[:, 0, None].to_broadcast([128, nidx]),
                        in1=dbc[:, :nidx],
                        op=mybir.AluOpType.is_equal,
                    )
                    ad_ps = psad.tile([128, CALL_TILES * H1], F32, tag="sm")
                    for t in range(nt):
                        nc.tensor.matmul(
                            out=ad_ps[:, t:t + 1],
                            lhsT=S[:, t * 128:(t + 1) * 128],
                            rhs=a2dres[:, b, None],
                            start=True, stop=True)
                    e4 = wk.tile([128, CALL_TILES * H1], F32, tag="e4")
                    nc.vector.tensor_add(
                        out=e4[:, :nt],
                        in0=G[:, :CALL_TILES * T2_COLS]
                            .rearrange("p (t c) -> p t c", c=T2_COLS)
                            [:, :nt, T2_AS],
                        in1=ad_ps[:, :nt],
                    )
                    lr = wk.tile([128, CALL_TILES * H1], F32, tag="lr")
                    nc.vector.scalar_tensor_tensor(
                        out=lr[:, :nt], in0=e4[:, :nt], scalar=NEG_SLOPE,
                        in1=e4[:, :nt],
                        op0=mybir.AluOpType.mult, op1=mybir.AluOpType.max)
                    w = wk.tile([128, CALL_TILES * H1], BF16, tag="w")
                    nc.scalar.activation(out=w[:, :nt], in_=lr[:, :nt],
                                         func=mybir.ActivationFunctionType.Exp)
                    WH = whpool.tile([128, CALL_TILES * (D1 + H1)], BF16,
                                     tag="WH")
                    WH2 = WH[:, :CALL_TILES * (D2 + 1)]
                    nc.vector.tensor_mul(
                        out=WH2.rearrange("p (t c) -> p t c", c=D2 + 1)
                            [:, :nt, :D2],
                        in0=G[:, :CALL_TILES * T2_COLS]
                            .rearrange("p (t c) -> p t c", c=T2_COLS)
                            [:, :nt, :D2],
                        in1=w[:, :nt, None].to_broadcast([128, nt, D2]),
                    )
                    nc.vector.tensor_copy(
                        out=WH2.rearrange("p (t c) -> p t c", c=D2 + 1)
                            [:, :nt, D2],
                        in_=w[:, :nt],
                    )
                    for t in range(nt):
                        nc.tensor.matmul(
                            out=out_ps[:],
                            lhsT=ST[:, t * 128:(t + 1) * 128],
                            rhs=WH2[:, t * (D2 + 1):(t + 1) * (D2 + 1)],
                            start=first and t == 0,
                            stop=(ci == total_calls - 1) and (t == nt - 1),
                        )
                    first = False
                    tile_idx += nt

                # ---- block epilogue: div, elu+1, pooling ----
                den = wk.tile([128, 1], F32, tag="den2")
                nc.vector.tensor_scalar_max(out=den[:], in0=out_ps[:, D2:],
                                            scalar1=1e-30)
                rec = wk.tile([128, 1], F32, tag="rec2")
                nc.vector.reciprocal(out=rec[:], in_=den[:])
                x2 = wk.tile([128, D2], F32, tag="x2")
                nc.scalar.activation(out=x2[:], in_=out_ps[:, :D2],
                                     func=mybir.ActivationFunctionType.Copy,
                                     scale=rec[:, 0, None])
                nc.vector.tensor_add(out=x2[:], in0=x2[:], in1=b2b_sb[:])
                u = wk.tile([128, D2], F32, tag="u2")
                nc.scalar.activation(out=u[:], in_=x2[:],
                                     func=mybir.ActivationFunctionType.Relu,
                                     scale=-1.0)
                tE = wk.tile([128, D2], F32, tag="tE2")
                nc.scalar.activation(out=tE[:], in_=u[:],
                                     func=mybir.ActivationFunctionType.Exp,
                                     scale=-1.0)
                o2y = wk.tile([128, D2 + 1], BF16, tag="o2y")
                nc.vector.scalar_tensor_tensor(
                    out=o2y[:, :D2], in0=x2[:], scalar=0.0, in1=tE[:],
                    op0=mybir.AluOpType.max, op1=mybir.AluOpType.add)
                nc.vector.memset(o2y[:, D2:], 1.0)
                pST = wk.tile([128, 128], BF16, tag="pST")
                nc.vector.tensor_tensor(
                    out=pST[:],
                    in0=bat_sb[:, b, None].to_broadcast([128, 128]),
                    in1=iot_sb[:],
                    op=mybir.AluOpType.is_equal,
                )
                nc.tensor.matmul(out=pool_ps[:], lhsT=pST[:], rhs=o2y[:],
                                 start=(b == 0), stop=(b == nblk - 1))

            # ================= P7: pool allreduce + tail =================
            pp = wk.tile([B, D2 + 1], F32, tag="pp")
            nc.vector.tensor_copy(out=pp[:], in_=pool_ps[:B])
            nc.sync.dma_start(out=prin[:, :], in_=pp[:])
            nc.gpsimd.collective_compute(
                "AllReduce", mybir.AluOpType.add,
                replica_groups=[list(range(NCORES))],
                ins=[prin.ap().opt()], outs=[prout.ap().opt()],
            )
            pr = wk.tile([B, D2 + 1], F32, tag="pr")
            nc.sync.dma_start(out=pr[:], in_=prout[:, :])
            cnt = wk.tile([B, 1], F32, tag="cnt")
            nc.vector.tensor_scalar_max(out=cnt[:], in0=pr[:, D2:],
                                        scalar1=1.0)
            rcnt = wk.tile([B, 1], F32, tag="rcnt")
            nc.vector.reciprocal(out=rcnt[:], in_=cnt[:])
            pooled = wk.tile([B, D2], F32, tag="pooled")
            nc.scalar.activation(out=pooled[:], in_=pr[:, :D2],
                                 func=mybir.ActivationFunctionType.Copy,
                                 scale=rcnt[:, 0, None])
            zT = wk.tile([128, B], F32, tag="zT")
            pT_ps = psad.tile([128, B], F32, tag="sm")
            nc.tensor.transpose(out=pT_ps[:D2, :B], in_=pooled[:],
                                identity=id_sb[:B, :B])
            nc.vector.tensor_copy(out=zT[:D2], in_=pT_ps[:D2, :B])
            # g-MLP (transposed layout: [feat, graph])
            gf_sb = wk.tile([32, B], F32, tag="gf")
            nc.sync.dma_start(out=gf_sb[:], in_=gfT[:])
            gw1_sb = wk.tile([32, D2], F32, tag="gw1")
            nc.sync.dma_start(out=gw1_sb[:], in_=gw1T[:])
            gb1_sb = wk.tile([D2, 1], F32, tag="gb1")
            nc.sync.dma_start(out=gb1_sb[:], in_=gb1c[:])
            gw2_sb = wk.tile([D2, D2], F32, tag="gw2")
            nc.sync.dma_start(out=gw2_sb[:], in_=gw2T[:])
            gb2_sb = wk.tile([D2, 1], F32, tag="gb2")
            nc.sync.dma_start(out=gb2_sb[:], in_=gb2c[:])
            fw_sb = wk.tile([2 * D2, 16], F32, tag="fw")
            nc.sync.dma_start(out=fw_sb[:], in_=fwT[:])
            fb_sb = wk.tile([B, 16], F32, tag="fb")
            nc.sync.dma_start(out=fb_sb[:], in_=fbb[:])

            g1_ps = psad.tile([D2, B], F32, tag="sm")
            nc.tensor.matmul(out=g1_ps[:], lhsT=gw1_sb[:], rhs=gf_sb[:],
                             start=True, stop=True)
            g1 = wk.tile([D2, B], F32, tag="g1")
            nc.scalar.activation(out=g1[:], in_=g1_ps[:],
                                 func=mybir.ActivationFunctionType.Relu,
                                 bias=gb1_sb[:, :1])
            g2_ps = psad.tile([D2, B], F32, tag="sm")
            nc.tensor.matmul(out=g2_ps[:], lhsT=gw2_sb[:], rhs=g1[:],
                             start=True, stop=True)
            nc.scalar.activation(out=zT[D2:2 * D2], in_=g2_ps[:],
                                 func=mybir.ActivationFunctionType.Relu,
                                 bias=gb2_sb[:, :1])
            fin_ps = psad.tile([B, 16], F32, tag="sm")
            nc.tensor.matmul(out=fin_ps[:], lhsT=zT[:, :B], rhs=fw_sb[:],
                             start=True, stop=True)
            fin = wk.tile([B, 16], F32, tag="fin")
            nc.vector.tensor_add(out=fin[:], in0=fin_ps[:], in1=fb_sb[:])
            nc.sync.dma_start(out=out[:, :], in_=fin[:])

    nc.compile()
    return nc


def _wrap_idx(idx_flat):
    """dma_gather idx layout: [128, n/16] int16, replicated per 16-row group."""
    n = len(idx_flat)
    a = np.empty((128, n // 16), dtype=np.int16)
    blk = idx_flat.reshape(n // 16, 16).T
    for g in range(8):
        a[g * 16:(g + 1) * 16, :] = blk
    return a


def host_prep(inputs, N, B, ncores=NCORES):
    """Sort/shard edges, build per-core tables. Returns (meta, in_maps)."""
    x = np.asarray(inputs["x"], np.float32)
    ei = np.asarray(inputs["edge_index"], np.int64)
    batch = np.asarray(inputs["batch"], np.int64)
    shard = N // ncores
    nblk = _ceil_div(shard, 128)
    shp = nblk * 128
    split = ((N // 2 + 127) // 128) * 128
    assert split < 32768

    loop = np.arange(N, dtype=np.int64)
    src = np.concatenate([ei[0], loop])
    dst = np.concatenate([ei[1], loop])
    order = np.argsort(dst, kind="stable")
    src, dst = src[order], dst[order]

    # layer-2 source rows live in agout: row = (s//shard)*shp + s%shard.
    # A/B split at asplit (first half of cores).
    asplit = (ncores // 2) * shp
    s_half = (ncores // 2) * shard     # nodes below this land in agout half A

    per_core = []
    nA1 = np.zeros((ncores, nblk), np.int64)
    nB1 = np.zeros((ncores, nblk), np.int64)
    nA2 = np.zeros((ncores, nblk), np.int64)
    nB2 = np.zeros((ncores, nblk), np.int64)
    for c in range(ncores):
        own0, own1 = c * shard, (c + 1) * shard
        sel = (dst >= own0) & (dst < own1)
        s_c, d_c = src[sel], dst[sel] - own0
        pos = np.where((s_c >= own0) & (s_c < own1), s_c - own0,
                       np.where(s_c < own0, s_c, s_c - shard) + shard)
        agr = (s_c // shard) * shp + (s_c % shard)
        blocks = []
        for b in range(nblk):
            lo = np.searchsorted(d_c, b * 128)
            hi = np.searchsorted(d_c, min((b + 1) * 128, shard))
            pb, gb, db = pos[lo:hi], agr[lo:hi], d_c[lo:hi] - b * 128
            mA = pb < split
            L1A = (pb[mA], db[mA])
            L1B = (pb[~mA] - split, db[~mA])
            gA = gb < asplit
            L2A = (gb[gA], db[gA])
            L2B = (gb[~gA] - asplit, db[~gA])
            blocks.append(((L1A, L1B), (L2A, L2B)))
            nA1[c, b], nB1[c, b] = len(L1A[0]), len(L1B[0])
            nA2[c, b], nB2[c, b] = len(L2A[0]), len(L2B[0])
        per_core.append(blocks)

    TA = [int(_ceil_div(int(nA1[:, b].max()), 128)) for b in range(nblk)]
    TB = [int(_ceil_div(int(nB1[:, b].max()), 128)) for b in range(nblk)]
    TA2 = [int(_ceil_div(int(nA2[:, b].max()), 128)) for b in range(nblk)]
    TB2 = [int(_ceil_div(int(nB2[:, b].max()), 128)) for b in range(nblk)]
    TT = sum(TA) + sum(TB) + sum(TA2) + sum(TB2)

    # per-core packed arrays
    in_cores = []
    for c in range(ncores):
        idx_cols, dst_cols = [], []
        for layer in (0, 1):
            Tmax = (TA, TB) if layer == 0 else (TA2, TB2)
            for b in range(nblk):
                (A, Bm) = per_core[c][b][layer]
                for (T, (p_arr, d_arr)) in ((Tmax[0][b], A), (Tmax[1][b], Bm)):
                    if T == 0:
                        continue
                    n = T * 128
                    pi = np.zeros(n, np.int16)
                    pi[:len(p_arr)] = p_arr.astype(np.int16)
                    di = np.full(n, 200.0, np.float32)
                    di[:len(d_arr)] = d_arr.astype(np.float32)
                    base = 0
                    for ct in _call_sizes(T):
                        idx_cols.append(_wrap_idx(pi[base:base + ct * 128]))
                        base += ct * 128
                    dst_cols.append(di.reshape(T, 128).T)
        idx16 = np.concatenate(idx_cols, axis=1)
        dstrel = np.concatenate(dst_cols, axis=1)
        dstT = np.ascontiguousarray(dstrel.T).reshape(1, -1)
        own0 = c * shard
        bat = np.full((128, nblk), 200.0, np.float32)
        for b in range(nblk):
            rows = min(128, shard - b * 128)
            bat[:rows, b] = batch[own0 + b * 128: own0 + b * 128 + rows]
        perm = np.concatenate([
            np.arange(own0, own0 + shard),
            np.arange(0, own0),
            np.arange(own0 + shard, N),
        ])
        in_cores.append({
            "idx16": idx16,
            "dstrel": dstrel.astype(ml_dtypes.bfloat16),
            "dstT": dstT.astype(ml_dtypes.bfloat16),
            "batchrel": bat.astype(ml_dtypes.bfloat16),
            "xT": np.ascontiguousarray(x[perm].T).astype(ml_dtypes.bfloat16),
        })

    # shared weights
    W1 = np.asarray(inputs["W1"], np.float32)       # [256, 128]
    a1s = np.asarray(inputs["a1s"], np.float32)     # [4, 64]
    a1d = np.asarray(inputs["a1d"], np.float32)
    b1 = np.asarray(inputs["b1"], np.float32)
    W2 = np.asarray(inputs["W2"], np.float32)       # [64, 256]
    a2s = np.asarray(inputs["a2s"], np.float32)     # [1, 64]
    a2d = np.asarray(inputs["a2d"], np.float32)
    b2 = np.asarray(inputs["b2"], np.float32)
    fw = np.asarray(inputs["fw"], np.float32)       # [16, 128]
    fb = np.asarray(inputs["fb"], np.float32)
    H1c, C1, Fin = a1s.shape[0], a1s.shape[1], W1.shape[1]
    W1r = W1.reshape(H1c, C1, Fin)
    Vs = np.einsum("hcf,hc->fh", W1r, a1s)
    Vd = np.einsum("hcf,hc->fh", W1r, a1d)
    w1text = np.concatenate([W1.T, Vs, Vd], axis=1).astype(np.float32)
    V2s = (a2s[0] @ W2)[:, None]
    V2d = (a2d[0] @ W2)[:, None]
    w2text = np.concatenate([W2.T, V2s, V2d], axis=1).astype(np.float32)
    w2bf = w2text.astype(ml_dtypes.bfloat16)
    # elu is computed as y=elu+1; fold the -1 through the next layer
    corr = -w2bf.astype(np.float32).sum(axis=0)     # [66]
    fb_adj = fb - np.asarray(fw, np.float32)[:, :64].sum(axis=1)
    shared = {
        "w1text": w1text.astype(ml_dtypes.bfloat16),
        "w2texta": w2bf[:128], "w2textb": w2bf[128:256],
        "corrb": np.tile(corr, (128, 1)).astype(np.float32),
        "b1b": np.tile(b1, (128, 1)).astype(np.float32),
        "b2b": np.tile(b2, (128, 1)).astype(np.float32),
        "ident": np.eye(128, dtype=np.float32),
        "iotar": np.tile(np.arange(128, dtype=np.float32),
                         (128, 1)).astype(ml_dtypes.bfloat16),
        "iotac": np.arange(128, dtype=np.float32)[:, None]
                   .astype(ml_dtypes.bfloat16),
        "gfT": np.ascontiguousarray(
            np.asarray(inputs["global_feats"], np.float32).T),
        "gw1T": np.ascontiguousarray(np.asarray(inputs["gw1"], np.float32).T),
        "gb1c": np.asarray(inputs["gb1"], np.float32)[:, None],
        "gw2T": np.ascontiguousarray(np.asarray(inputs["gw2"], np.float32).T),
        "gb2c": np.asarray(inputs["gb2"], np.float32)[:, None],
        "fwT": np.ascontiguousarray(fw.T),
        "fbb": np.tile(fb_adj, (B, 1)).astype(np.float32),
    }
    in_maps = [{**shared, **pc} for pc in in_cores]
    meta = dict(N=N, B=B, TA=TA, TB=TB, TA2=TA2, TB2=TB2, nblk=nblk,
                shard=shard, split=split, TT=TT)
    return meta, in_maps


_PROGRAM_CACHE = {}


def run(inputs, N, B, trace=False):
    meta, in_maps = host_prep(inputs, N, B)
    key = (N, B, tuple(meta["TA"]), tuple(meta["TB"]),
           tuple(meta["TA2"]), tuple(meta["TB2"]))
    nc = _PROGRAM_CACHE.get(key)
    if nc is None:
        nc = build_program(N, B, meta["TA"], meta["TB"], meta["TA2"],
                           meta["TB2"], meta["nblk"], meta["shard"],
                           meta["split"])
        _PROGRAM_CACHE[key] = nc
    res = bass_utils.run_bass_kernel_spmd(
        nc, in_maps, core_ids=list(range(NCORES)), trace=trace)
    return np.asarray(res.results[0]["out"]), res


def kernel(**inputs) -> np.ndarray:
    N = int(np.asarray(inputs["x"]).shape[0])
    B = int(np.asarray(inputs["global_feats"]).shape[0])
    out, _ = run(inputs, N, B)
    return out
